# revision 14
# baseline (speedup 1.0000x reference)
"""Distributed Trainium2 kernel for nn_Attention_31104153157828.

Computation (B=16, S=2048, D=1024):
    fac1 = k @ W                     [B,S,D]
    fac2 = (q @ U)[:, None, :]       [B,1,D]
    t    = tanh(fac1 + fac2)
    s    = einsum('bsd,bse->bde', v, t)      [B,D,D]
    attn = softmax(s, axis=0)                 (softmax over BATCH)
    out  = einsum('bsd,bde->bse', v, attn)   [B,S,D]

Sharding: data-parallel over batch, 2 batches per core on 8 cores.
The batch-axis softmax needs a cross-core AllReduce of max and sum(exp)
over the [D,D] logit matrix, pipelined by e-half so the AllReduces hide
under tensor-engine work.

v3 design:
  - All matmuls fp16; PSUM/logits f32. k cast to DRAM fp16, kT and vT
    tiles via HWDGE xbar-transpose loads (no PE transposes).
  - fac2 bias folded in via one DVE add on the PSUM tile per (m,h)
    instead of 64 rank-1 matmuls.
  - In-order queue discipline (no AR-gated op ahead of critical work):
      PE:     fac2 MMs, A MMs, B MMs, C MMs
      Scalar: tanh, exp, Ln/Exp(-x) 1/Z, mx/sm bounce STORES
      Vector: fac2 bcast evict, A bias adds, B/C PSUM evicts, softmax
              max/sub/add/mul (interleaved so C evicts never starve)
      GpSimd: casts, warm+real AR triggers, AR-gated bounce LOADS
      Sync:   kT/vT xbar transposes, v slabs, C out-stores
  - Warmup collective split: AR_w1 early, AR_w2 after the casts, so the
    gpsimd queue never blocks the k16/v16/W16 loads.
"""
import numpy as np
import concourse.bass as bass
import concourse.bacc as bacc
import concourse.tile as tile
import concourse.mybir as mybir
from concourse.bass_utils import run_bass_kernel_spmd

F32 = mybir.dt.float32
F32R = mybir.dt.float32r
F16 = mybir.dt.float16
AF = mybir.ActivationFunctionType

B, S, D = 16, 2048, 1024
N_CORES = 8
BL = B // N_CORES          # local batches per core = 2
M_T = S // 128             # 16 s-tiles
KC = D // 128              # 8 contraction chunks (d)
EH = 2                     # e halves of 512
ARC = 4                    # softmax chunks (pairs of d-tiles)
MG = 8                     # m-tiles per kT transpose group (stage A)
CK = 4                     # m-tiles per k16 cast chunk
RG = [list(range(N_CORES))]


def build():
    nc = bacc.Bacc("TRN2", target_bir_lowering=False, debug=False,
                   num_devices=N_CORES)

    q2 = nc.dram_tensor("q2", [BL, D], F32, kind="ExternalInput")
    k2 = nc.dram_tensor("k2", [BL, S, D], F32, kind="ExternalInput")
    v2 = nc.dram_tensor("v2", [BL, S, D], F32, kind="ExternalInput")
    Wd = nc.dram_tensor("W", [D, D], F32, kind="ExternalInput")
    Ud = nc.dram_tensor("U", [D, D], F32, kind="ExternalInput")
    out2 = nc.dram_tensor("out", [BL, S, D], F32, kind="ExternalOutput")

    k16 = nc.dram_tensor("k16", [BL, S, D], F16)
    v16 = nc.dram_tensor("v16", [BL, S, D], F16)

    # collective bounce buffers, one set per e-half
    mx_in = [nc.dram_tensor(f"mx_in{h}", [128, KC, 512], F16) for h in range(EH)]
    mx_out = [nc.dram_tensor(f"mx_out{h}", [128, KC, 512], F16) for h in range(EH)]
    sm_in = [nc.dram_tensor(f"sm_in{h}", [128, KC, 512], F16) for h in range(EH)]
    sm_out = [nc.dram_tensor(f"sm_out{h}", [128, KC, 512], F16) for h in range(EH)]

    warm_in = nc.dram_tensor("warm_in", [128, 16], F32)
    warm_out = nc.dram_tensor("warm_out", [128, 16], F32)
    warm_out2 = nc.dram_tensor("warm_out2", [128, 16], F32)

    warm_d = nc.inline_tensor(np.ones((128, 16), np.float32), name="warm_d")
    ones_d = nc.inline_tensor(np.ones((1, 128), np.float32), name="ones1")

    with tile.TileContext(nc) as tc:
        with tc.tile_pool(name="rp", bufs=1) as rp:
            # ---- long-lived pools first (LIFO close order) ----
            cp_cm = tc.tile_pool(name="cp", bufs=3)
            cp = cp_cm.__enter__()
            smx_cm = tc.tile_pool(name="smx", bufs=2)
            smx = smx_cm.__enter__()
            rbp_cm = tc.tile_pool(name="rbp", bufs=4)
            rbp = rbp_cm.__enter__()
            tp_cm = tc.tile_pool(name="tp", bufs=1)
            tp = tp_cm.__enter__()
            t_sb = [tp.tile([128, M_T, D], F16, name=f"t{b}") for b in range(BL)]

            wp_cm = tc.tile_pool(name="wp", bufs=1)
            wp = wp_cm.__enter__()

            # warmup collective part 1 (fires while casts stream)
            wtile = rp.tile([128, 16], F32, name="wtile")
            nc.gpsimd.dma_start(wtile[:], warm_d.ap())
            nc.gpsimd.dma_start(warm_in.ap(), wtile[:])

            # first k casts so stage A's transposes can start ASAP
            for mg in range(0, 8, CK):
                nc.gpsimd.dma_start(
                    k16.ap()[0, mg * 128:(mg + CK) * 128, :],
                    k2.ap()[0, mg * 128:(mg + CK) * 128, :])
            W16 = wp.tile([128, KC, D], F16, name="W16")
            fac2b = wp.tile([128, BL, D], F32, name="fac2b")
            U16 = wp.tile([128, KC, D], F16, name="U16")
            # W/U ride the sync HWDGE queue as f32 + DVE convert, keeping
            # the gpsimd SWDGE queue free for the k/v casts (one shared
            # f32 scratch slot, W then U)
            with tc.tile_pool(name="wl", bufs=1) as wlp:
                Wf = wlp.tile([128, KC, D], F32, tag="wl", name="Wf")
                nc.sync.dma_start(
                    Wf[:], Wd.ap().rearrange("(kc p) e -> p kc e", p=128))
                nc.vector.tensor_copy(W16[:], Wf[:])
                Uf = wlp.tile([128, KC, D], F32, tag="wl", name="Uf")
                nc.sync.dma_start(
                    Uf[:], Ud.ap().rearrange("(kc p) e -> p kc e", p=128))
                nc.vector.tensor_copy(U16[:], Uf[:])

            # ---- fac2 = q @ U (f32r), broadcast to 128 partitions ----
            with (
                tc.tile_pool(name="f2", bufs=1) as f2p,
                tc.tile_pool(name="f2u", bufs=1) as f2u,
                tc.tile_pool(name="f2ps", bufs=2, space="PSUM") as f2ps,
            ):
                ones16 = f2p.tile([1, 128], F16, tag="on", name="ones16")
                nc.gpsimd.dma_start(ones16[:], ones_d.ap())
                fac2r = f2p.tile([1, BL, D], F16, tag="fr", name="fac2r")
                U_r = U16
                qcols = []
                for b in range(BL):
                    qcol = f2p.tile([128, KC], F16, tag="qcol", name=f"qcol{b}")
                    nc.gpsimd.dma_start(
                        qcol[:], q2.ap()[b].rearrange("(kc p) -> p kc", p=128))
                    qcols.append(qcol)
                for mg in range(8, M_T, CK):
                    nc.gpsimd.dma_start(
                        k16.ap()[0, mg * 128:(mg + CK) * 128, :],
                        k2.ap()[0, mg * 128:(mg + CK) * 128, :])
                ar_w1 = nc.gpsimd.collective_compute(
                    "AllReduce", mybir.AluOpType.max, replica_groups=RG,
                    ins=[warm_in.ap().opt()], outs=[warm_out.ap().opt()])
                # rest of the casts; AR_w2 last so its wait blocks nothing
                for mg in range(0, M_T, CK):
                    nc.gpsimd.dma_start(
                        k16.ap()[1, mg * 128:(mg + CK) * 128, :],
                        k2.ap()[1, mg * 128:(mg + CK) * 128, :])
                for b in range(BL):
                    nc.gpsimd.dma_start(v16.ap()[b], v2.ap()[b])
                ar_w2 = nc.gpsimd.collective_compute(
                    "AllReduce", mybir.AluOpType.add, replica_groups=RG,
                    ins=[warm_out.ap().opt()], outs=[warm_out2.ap().opt()])

                for b in range(BL):
                    for h in range(EH):
                        ps = f2ps.tile([1, 512], F32, tag="f2ps",
                                       name=f"f2ps{b}_{h}")
                        for kc in range(KC):
                            nc.tensor.matmul(ps[:], qcols[b][:, kc:kc + 1],
                                             U_r[:, kc, h * 512:(h + 1) * 512],
                                             start=(kc == 0), stop=(kc == KC - 1))
                        nc.scalar.copy(fac2r[0:1, b, h * 512:(h + 1) * 512], ps[:])
                # broadcast fac2 along partitions: ones^T @ fac2r
                for b in range(BL):
                    for h in range(EH):
                        ps = f2ps.tile([128, 512], F32, tag="f2bc",
                                       name=f"f2bc{b}_{h}")
                        nc.tensor.matmul(ps[:], ones16[:],
                                         fac2r[0:1, b, h * 512:(h + 1) * 512],
                                         start=True, stop=True)
                        nc.vector.tensor_copy(fac2b[:, b, h * 512:(h + 1) * 512],
                                              ps[:])

            # ======== stage A: t = tanh(k @ W + fac2), fp16 ========
            with (
                tc.tile_pool(name="akt", bufs=2) as ktp,
                tc.tile_pool(name="aps", bufs=2, space="PSUM") as aps,
            ):
                groups = {0: [(0, 4), (4, 4), (8, 8)], 1: [(0, 8), (8, 8)]}
                for b in range(BL):
                    for mg, glen in groups[b]:
                        ktg = ktp.tile([128, KC, glen * 128], F16, tag="ktg",
                                       name=f"ktg{b}_{mg}")
                        nc.sync.dma_start(
                            ktg[:], k16.ap()[b, mg * 128:(mg + glen) * 128, :],
                            transpose=True)
                        for j in range(glen):
                            m = mg + j
                            psh = [aps.tile([128, 512], F32, tag=f"aps{h}",
                                            name=f"aps{b}_{m}_{h}")
                                   for h in range(EH)]
                            for kc in range(KC):
                                for h in range(EH):
                                    nc.tensor.matmul(
                                        psh[h][:],
                                        ktg[:, kc, j * 128:(j + 1) * 128],
                                        W16[:, kc, h * 512:(h + 1) * 512],
                                        start=(kc == 0), stop=(kc == KC - 1))
                            for h in range(EH):
                                nc.vector.tensor_add(
                                    psh[h][:], psh[h][:],
                                    fac2b[:, b, h * 512:(h + 1) * 512])
                                nc.scalar.activation(
                                    t_sb[b][:, m, h * 512:(h + 1) * 512],
                                    psh[h][:], AF.Tanh)

            wp_cm.__exit__(None, None, None)

            # ======== stages B + softmax + C, pipelined by e-half ========
            sp_cm = tc.tile_pool(name="sp", bufs=2)
            sp = sp_cm.__enter__()
            bp_cm = tc.tile_pool(name="bp", bufs=6)
            bp = bp_cm.__enter__()
            bps_cm = tc.tile_pool(name="bps", bufs=1, space="PSUM")
            bps = bps_cm.__enter__()

            def stage_b_batch(h, b):
                psb = [bps.tile([128, 512], F32, tag=f"pb{dt}",
                                name=f"pb{h}_{b}_{dt}") for dt in range(KC)]
                for m in range(M_T):
                    vslab = bp.tile([128, D], F16, tag="vslab",
                                    name=f"vslab{h}_{b}_{m}")
                    nc.sync.dma_start(
                        vslab[:], v16.ap()[b, m * 128:(m + 1) * 128, :])
                    for dt in range(KC):
                        nc.tensor.matmul(
                            psb[dt][:],
                            vslab[:, dt * 128:(dt + 1) * 128],
                            t_sb[b][:, m, h * 512:(h + 1) * 512],
                            start=(m == 0), stop=(m == M_T - 1))
                s_b = sp.tile([128, KC, 512], F16, tag=f"s{b}", name=f"s{h}_{b}")
                for dt in range(KC):
                    nc.vector.tensor_copy(s_b[:, dt, :], psb[dt][:])
                return s_b

            def vtg_load(h, b, mg):
                vtg = cp.tile([128, KC, 1024], F16, tag="vtg",
                              name=f"vtg{h}_{b}_{mg}")
                nc.scalar.dma_start(
                    vtg[:], v16.ap()[b, mg * 128:(mg + 8) * 128, :],
                    transpose=True)
                return vtg

            def local_max(h, s_h):
                for c in range(ARC):
                    dsl = slice(2 * c, 2 * c + 2)
                    mx = smx.tile([128, 2, 512], F16, tag="sfb", name=f"mx{h}_{c}")
                    nc.vector.tensor_max(mx[:], s_h[0][:, dsl, :],
                                         s_h[1][:, dsl, :])
                    nc.scalar.dma_start(mx_in[h].ap()[:, dsl, :], mx[:])

            def exp_and_sum(h, s_h):
                # after AR-max(h): subtract gmax, exp IN PLACE (p == s
                # tile, so no separate p pool), local sum
                for c in range(ARC):
                    dsl = slice(2 * c, 2 * c + 2)
                    gmxb = smx.tile([128, 2, 512], F16, tag="sfb",
                                    name=f"gmxb{h}_{c}")
                    nc.gpsimd.dma_start(gmxb[:], mx_out[h].ap()[:, dsl, :])
                    for b in range(BL):
                        nc.vector.tensor_sub(s_h[b][:, dsl, :],
                                             s_h[b][:, dsl, :], gmxb[:])
                        nc.scalar.activation(s_h[b][:, dsl, :],
                                             s_h[b][:, dsl, :], AF.Exp)
                    sm = smx.tile([128, 2, 512], F16, tag="sfb", name=f"sm{h}_{c}")
                    nc.vector.tensor_add(sm[:], s_h[0][:, dsl, :],
                                         s_h[1][:, dsl, :])
                    nc.scalar.dma_start(sm_in[h].ap()[:, dsl, :], sm[:])

            def recip_z(h):
                # after AR-sum(h): recb = 1/Z = exp(-ln(Z)) on scalar
                recbs = []
                for c in range(ARC):
                    dsl = slice(2 * c, 2 * c + 2)
                    zz = smx.tile([128, 2, 512], F16, tag="sfb", name=f"zz{h}_{c}")
                    nc.gpsimd.dma_start(zz[:], sm_out[h].ap()[:, dsl, :])
                    rec = smx.tile([128, 2, 512], F16, tag="sfb",
                                   name=f"rec{h}_{c}")
                    nc.scalar.activation(rec[:], zz[:], AF.Ln)
                    recb = rbp.tile([128, 2, 512], F16, tag="recb",
                                    name=f"recb{h}_{c}")
                    nc.scalar.activation(recb[:], rec[:], AF.Exp, scale=-1.0)
                    recbs.append(recb)
                return recbs

            def attn_mul(h, p_h, recbs):
                for c in range(ARC):
                    dsl = slice(2 * c, 2 * c + 2)
                    for b in range(BL):
                        nc.vector.tensor_mul(p_h[b][:, dsl, :],
                                             p_h[b][:, dsl, :], recbs[c][:])

            vtgs = {}
            p_t = {}
            s00 = stage_b_batch(0, 0)
            s01 = stage_b_batch(0, 1)
            s0 = [s00, s01]
            p_t[0] = s0
            local_max(0, s0)
            vtgs[(0, 0, 0)] = vtg_load(0, 0, 0)
            vtgs[(0, 0, 8)] = vtg_load(0, 0, 8)
            ar_mx0 = nc.gpsimd.collective_compute(
                "AllReduce", mybir.AluOpType.max, replica_groups=RG,
                ins=[mx_in[0].ap().opt()], outs=[mx_out[0].ap().opt()])
            tile.add_dep_helper(ar_mx0.ins, ar_w2.ins, sync=False,
                                reason="serialize collectives")

            # h0 softmax enqueued BEFORE stage_b(1): its vector ops wait
            # only on AR-max(h0), which lands while B(h1) computes.
            exp_and_sum(0, s0)
            ar_sm0 = nc.gpsimd.collective_compute(
                "AllReduce", mybir.AluOpType.add, replica_groups=RG,
                ins=[sm_in[0].ap().opt()], outs=[sm_out[0].ap().opt()])
            tile.add_dep_helper(ar_sm0.ins, ar_mx0.ins, sync=False,
                                reason="serialize collectives")

            vtgs[(0, 1, 0)] = vtg_load(0, 1, 0)
            vtgs[(0, 1, 8)] = vtg_load(0, 1, 8)
            s10 = stage_b_batch(1, 0)
            s11 = stage_b_batch(1, 1)
            s1 = [s10, s11]
            p_t[1] = s1
            local_max(1, s1)
            ar_mx1 = nc.gpsimd.collective_compute(
                "AllReduce", mybir.AluOpType.max, replica_groups=RG,
                ins=[mx_in[1].ap().opt()], outs=[mx_out[1].ap().opt()])
            tile.add_dep_helper(ar_mx1.ins, ar_sm0.ins, sync=False,
                                reason="serialize collectives")

            recb0 = recip_z(0)
            attn_mul(0, p_t[0], recb0)

            bps_cm.__exit__(None, None, None)
            bp_cm.__exit__(None, None, None)

            op_cm = tc.tile_pool(name="op", bufs=12)
            op = op_cm.__enter__()
            cps_cm = tc.tile_pool(name="cps", bufs=2, space="PSUM")
            cps = cps_cm.__enter__()

            def stage_c_round(h, b, mg):
                # two sub-rounds of 4 m-tiles on alternating PSUM bank
                # sets (tags cps0-3, bufs=2): evictions of one sub-round
                # overlap the next sub-round's matmuls
                vtg = vtgs[(h, b, mg)]
                for half in range(2):
                    j0 = half * 4
                    pss = [cps.tile([128, 512], F32, tag=f"cps{j}",
                                    name=f"cps{h}_{b}_{mg}_{j0 + j}")
                           for j in range(4)]
                    for kc in range(KC):
                        for j in range(4):
                            nc.tensor.matmul(
                                pss[j][:],
                                vtg[:, kc, (j0 + j) * 128:(j0 + j + 1) * 128],
                                p_t[h][b][:, kc, :],
                                start=(kc == 0), stop=(kc == KC - 1))
                    for j in range(4):
                        m = mg + j0 + j
                        ost = op.tile([128, 512], F32, tag="ost",
                                      name=f"ost{h}_{b}_{m}")
                        nc.vector.tensor_copy(ost[:], pss[j][:])
                        nc.sync.dma_start(
                            out2.ap()[b, m * 128:(m + 1) * 128,
                                      h * 512:(h + 1) * 512], ost[:])

            stage_c_round(0, 0, 0)
            # h1 softmax rides between C(h0) rounds; its vector ops are
            # enqueued after round 1's evictions so they never starve C.
            exp_and_sum(1, s1)
            ar_sm1 = nc.gpsimd.collective_compute(
                "AllReduce", mybir.AluOpType.add, replica_groups=RG,
                ins=[sm_in[1].ap().opt()], outs=[sm_out[1].ap().opt()])
            tile.add_dep_helper(ar_sm1.ins, ar_mx1.ins, sync=False,
                                reason="serialize collectives")
            vtgs[(1, 0, 0)] = vtg_load(1, 0, 0)
            stage_c_round(0, 0, 8)
            vtgs[(1, 0, 8)] = vtg_load(1, 0, 8)
            stage_c_round(0, 1, 0)
            recb1 = recip_z(1)
            vtgs[(1, 1, 0)] = vtg_load(1, 1, 0)
            stage_c_round(0, 1, 8)
            attn_mul(1, p_t[1], recb1)
            vtgs[(1, 1, 8)] = vtg_load(1, 1, 8)

            stage_c_round(1, 0, 0)
            stage_c_round(1, 0, 8)
            stage_c_round(1, 1, 0)
            stage_c_round(1, 1, 8)

            cps_cm.__exit__(None, None, None)
            op_cm.__exit__(None, None, None)
            sp_cm.__exit__(None, None, None)
            tp_cm.__exit__(None, None, None)
            rbp_cm.__exit__(None, None, None)
            smx_cm.__exit__(None, None, None)
            cp_cm.__exit__(None, None, None)

    nc.compile()
    return nc


_NC = None


def _get_nc():
    global _NC
    if _NC is None:
        _NC = build()
    return _NC


def kernel(q, k, v, W, U):
    q = np.ascontiguousarray(np.asarray(q, dtype=np.float32))
    k = np.ascontiguousarray(np.asarray(k, dtype=np.float32))
    v = np.ascontiguousarray(np.asarray(v, dtype=np.float32))
    W = np.ascontiguousarray(np.asarray(W, dtype=np.float32))
    U = np.ascontiguousarray(np.asarray(U, dtype=np.float32))

    nc = _get_nc()
    in_maps = [
        {
            "q2": q[c * BL:(c + 1) * BL],
            "k2": k[c * BL:(c + 1) * BL],
            "v2": v[c * BL:(c + 1) * BL],
            "W": W,
            "U": U,
        }
        for c in range(N_CORES)
    ]
    res = run_bass_kernel_spmd(nc, in_maps, core_ids=list(range(N_CORES)))
    out = np.concatenate([res.results[c]["out"] for c in range(N_CORES)], axis=0)
    return out.astype(np.float32)


if __name__ == "__main__":
    rng = np.random.default_rng(0)
    q = rng.standard_normal((B, D), dtype=np.float32)
    k = rng.standard_normal((B, S, D), dtype=np.float32)
    v = rng.standard_normal((B, S, D), dtype=np.float32)
    W = (rng.standard_normal((D, D), dtype=np.float32) / np.sqrt(D)).astype(np.float32)
    U = (rng.standard_normal((D, D), dtype=np.float32) / np.sqrt(D)).astype(np.float32)
    out = kernel(q=q, k=k, v=v, W=W, U=U)
    print("out", out.shape, out.dtype, float(np.abs(out).mean()))
